# revision 58
# baseline (speedup 1.0000x reference)
"""BrushStroke splat kernel for 8 trn2 NeuronCores (v3).

out[b,c,y,x] = mean_n sum_{p,q} Fy[b,n,y,p] Fx[b,n,x,q] patches[b,n,c,p,q]
with Fx/Fy separable Gaussian filter banks (sigma=0.1) normalized over a
padded spatial axis.

Strategy (per core, 2 batches of 64 strokes), batch-parallel, no
collectives:
 - Filters are generated DIRECTLY in the per-group [(j,q'), x] layout the
   PE needs: one Derivative_Erf activation per (group, coord) over a
   constant index tile I[(j,q'), x] = q' + x with a per-partition bias
   -(g_stroke + center)/sqrt(SIGMA2).  DErf(x) = k*exp(-x^2); the
   constant k cancels in the window normalization (DVE reduce).
 - The 128x64 bias table is built on-chip with one masked f32 matmul from
   the normalized brush coords.
 - All matmuls bf16: MM1 lhsT = block-diag patches, rhs = Fx-norm -> t in
   one [128,384] PSUM tile, one cast to bf16; MM2 weights = Fy-norm
   slices shared across channels (c0,c1 fused free=2W; redundant
   LDWEIGHTS deduped post-legalize), accumulated over 16 stroke groups.
 - Windowed mode: since the output is invariant under permuting the
   stroke dim, the host assigns strokes to groups so group g's x-support
   fits a STATIC 128-wide window at x0(g) (same across cores -> SPMD ok).
   MM1 free=128, MM2 writes strided [c0|c1] column windows of the acc
   (groups 0/15 own disjoint halves and run first with start=True).
   Falls back to the full-width program if assignment is infeasible.
"""
import os, sys, types
import numpy as np

IMAGE = 256
PAD = 16
EPS = 1e-7
SIGMA2 = 2.0 * 0.1 ** 2
RSIG = 1.0 / np.sqrt(SIGMA2)    # 7.0710678
B, N, C, PH, PW = 16, 64, 3, 32, 32
NCORES = 8
BLOC = B // NCORES          # 2 batches per core
NG = N // 4                 # 16 groups of 4 strokes
CX = PW / 2 - 0.5 + PAD     # 31.5
CY = PW / 2 - 0.4 + PAD     # 31.6

# static per-group x windows (windowed mode); edge-stacked so the
# left/right tails (strokes only coverable by an edge window) get 16 slots
X0 = [0, 0, 0, 0, 14, 28, 43, 57, 71, 85, 100, 114, 128, 128, 128, 128]
V0 = [min(max(x - 16, 0), 112) for x in X0]
WGENX = 176                 # x generation window (windowed mode)
WF = 128                    # x filter/image window (windowed mode)
WGENY = 288                 # y generation window (always full)


def _install_patches():
    if 'antenv.axon_hooks' not in sys.modules:
        mod = types.ModuleType('antenv.axon_hooks')
        mod._hook = None
        mod.set_axon_ntff_profile_hook = lambda h: setattr(mod, '_hook', h)
        mod.get_axon_ntff_profile_hook = lambda: mod._hook
        sys.modules['antenv.axon_hooks'] = mod
        try:
            from trn_agent_boot.trn_boot import _ntff_profile_via_ctypes
            hook = _ntff_profile_via_ctypes('/opt/axon/libaxon_pjrt.so')
            if hook is not None:
                mod.set_axon_ntff_profile_hook(hook)
        except Exception:
            pass

    import concourse.tile as tile
    import concourse.bass_utils as bass_utils
    from concourse.vector_clock import ScopedClock

    bass_utils.upload_artifacts = lambda tmpdir: 'local://' + tmpdir

    if getattr(tile.TileContext._drain_and_barrier, '_patched', False):
        return

    def _drain_and_barrier(self, tick_clock, wait_clock):
        nc = self.nc
        drain_inst = nc.sync.drain()
        wait_clock.add_sem_waits(
            drain_inst.ins, ScopedClock({None: tick_clock.global_clock}))
        si = drain_inst.ins.sync_info
        waits = list(si.on_wait or [])
        si.on_wait = []
        for w in waits:
            nop = nc.sync.nop()
            nop.ins.sync_info = type(si)(on_wait=[w], on_update=[])
        nc.all_engine_barrier()
        popped = nc._tile_sem_poison_stack.pop()
        assert popped is self._sem_poison
        nc.clear_and_free_semaphores(list(self.sems.allocated().values()))
        nc.all_engine_barrier()

    _drain_and_barrier._patched = True
    tile.TileContext._drain_and_barrier = _drain_and_barrier


def _split_multi_waits(nc):
    """This walrus accepts at most one sync wait per instruction; hoist
    extras onto same-engine NoOps inserted just before."""
    import bass_rust
    n_new = [0]

    def fresh_nop(engine, wait, si_type):
        n_new[0] += 1
        nop = bass_rust.InstNoOp(name=f'I-waitsplit-{n_new[0]}', ins=[], outs=[])
        nop.engine = engine
        nop.sync_info = si_type(on_wait=[wait], on_update=[])
        return nop

    for fn in nc.m.functions:
        for blk in fn.blocks:
            insts = blk.instructions
            i = 0
            while i < len(insts):
                inst = insts[i]
                si = inst.sync_info
                if si is not None and si.on_wait and len(si.on_wait) > 1:
                    waits = list(si.on_wait)
                    si.on_wait = [waits[-1]]
                    for k, w in enumerate(waits[:-1]):
                        insts.insert(i + k, fresh_nop(inst.engine, w, type(si)))
                    i += len(waits) - 1
                i += 1


def _dedupe_ldweights(nc):
    """Drop an InstLdweights whose weights AP is identical to the previous
    LDWEIGHTS on the stream (PE retains loaded weights); preserve its
    waits/updates on a PE NoOp."""
    import bass_rust
    n_del = [0]
    for fn in nc.m.functions:
        for blk in fn.blocks:
            out = []
            last_key = None
            for inst in blk.instructions:
                tn = type(inst).__name__
                if tn == 'InstLdweights':
                    key = repr(inst.ins[0])
                    if key == last_key:
                        si = inst.sync_info
                        if si is not None and (si.on_wait or si.on_update):
                            nop = bass_rust.InstNoOp(
                                name=f'I-ldwdedup-{n_del[0]}', ins=[], outs=[])
                            nop.engine = inst.engine
                            nop.sync_info = si
                            out.append(nop)
                        n_del[0] += 1
                        continue
                    last_key = key
                out.append(inst)
            blk.instructions = out
    return n_del[0]


_PROGRAMS = {}


def _build_program(windowed):
    if windowed in _PROGRAMS:
        return _PROGRAMS[windowed]
    _install_patches()
    import concourse.bass as bass
    import concourse.tile as tile
    from concourse import mybir
    from bass_rust import AP

    f32 = mybir.dt.float32
    bf16 = mybir.dt.bfloat16
    AF = mybir.ActivationFunctionType
    AX = mybir.AxisListType

    wgx = WGENX if windowed else WGENY  # x generation width: 192 or 288
    wf = WF if windowed else IMAGE      # image window per group
    x0s = X0 if windowed else [0] * NG
    v0s = V0 if windowed else [0] * NG
    # fxn slice start inside the gen tile: image x0 <-> gen col x0+16-v0
    sxs = [x0s[g] + PAD - v0s[g] for g in range(NG)]
    wgen = max(wgx, WGENY)

    nc = bass.Bass('TRN2', target_bir_lowering=False, debug=False,
                   num_devices=NCORES)
    pt_in = nc.declare_dram_parameter('pt_in', [BLOC, 128, NG * C * PH],
                                      bf16, isOutput=False)
    itile = nc.declare_dram_parameter('itile', [128, wgen], f32,
                                      isOutput=False)
    bias_in = nc.declare_dram_parameter('bias_in', [128, 4 * NG], f32,
                                        isOutput=False)
    y_out = nc.declare_dram_parameter('y_out', [BLOC, C, IMAGE, IMAGE], f32,
                                      isOutput=True)
    debug = bool(os.environ.get('BRUSH_DEBUG'))
    if debug:
        dbg_bias = nc.declare_dram_parameter('dbg_bias', [128, 4 * NG], f32,
                                             isOutput=True)
        dbg_ws = nc.declare_dram_parameter('dbg_ws', [128, 4 * NG], f32,
                                           isOutput=True)
        dbg_fex = nc.declare_dram_parameter('dbg_fex', [128, 2 * WGENY], f32,
                                            isOutput=True)
        dbg_tt = nc.declare_dram_parameter('dbg_tt', [128, 3 * 256], f32,
                                           isOutput=True)

    with tile.TileContext(nc) as tc:
        with tc.tile_pool(name='glob', bufs=1) as gp:
            # ---- const loads (bias table is host-computed coordinate
            # preprocessing, like the stroke->group permutation) ----
            it = gp.tile([128, wgen], f32)
            nc.sync.dma_start(it[:], itile[:])     # gates the first act
            bias_a = gp.tile([128, 4 * NG], f32)
            nc.scalar.dma_start(bias_a[:], bias_in[:])
            # patches: 4 row-band DMAs per batch so each block-diag copy
            # starts as soon as its band lands (batch 0 first)
            ptc = []
            ps_all = []
            for b in range(BLOC):
                p = gp.tile([128, NG * C * PH], bf16, name=f'ptc{b}',
                            tag=f'ptc{b}')
                ptc.append(p)
                pa = gp.tile([128, 128 * C * NG], bf16, name=f'psall{b}',
                             tag=f'psall{b}')
                nc.gpsimd.memset(pa.bitcast(f32)[:], 0.0)
                ps_all.append(pa)

            def load_patches(b):
                for j in range(4):
                    rs = slice(32 * j, 32 * j + 32)
                    nc.sync.dma_start(ptc[b][rs, :], pt_in[b, rs, :])

            def blockdiag(b):
                for j in range(4):
                    dst0 = ps_all[b][32 * j:32 * j + 1, 32 * j:32 * j + 1]
                    dst = AP(ps_all[b].tensor, dst0.offset,
                             [[128 * C * NG, 32], [128 * C, NG],
                              [128, C], [1, PH]])
                    src0 = ptc[b][32 * j:32 * j + 1, 0:1]
                    srcap = AP(ptc[b].tensor, src0.offset,
                               [[NG * C * PH, 32], [C * PH, NG],
                                [PH, C], [1, PH]])
                    nc.vector.tensor_copy(dst, srcap)

            load_patches(0)
            blockdiag(0)
            if debug:
                nc.sync.dma_start(dbg_bias[:], bias_a[:])

            ws_all = gp.tile([128, 4 * NG], f32)
            inv_all = gp.tile([128, 4 * NG], f32)
            fe = {}

            def gen_acts(b):
                for g in range(NG):
                    for coord in range(2):
                        col = NG * (2 * b + coord) + g   # bias layout
                        w = wgx if coord == 0 else WGENY
                        f = gp.tile([128, w], bf16, name=f'fe{b}_{g}_{coord}',
                                    tag=f'fe{b}_{g}_{coord}')
                        # x window sums ride the activation accumulator
                        # (Scalar has headroom, DVE is saturated)
                        acc = (ws_all[:, 2 * NG * b + 2 * g:
                                      2 * NG * b + 2 * g + 1]
                               if coord == 0 else None)
                        nc.scalar.activation(f[:], it[:, 0:w],
                                             AF.Derivative_Erf,
                                             bias=bias_a[:, col:col + 1],
                                             scale=RSIG, accum_out=acc)
                        fe[(b, g, coord)] = f
                        if debug and b == 0 and g == 5:
                            dfx = gp.tile([128, w], f32, name=f'dfx{coord}',
                                          tag=f'dfx{coord}')
                            nc.vector.tensor_copy(dfx[:], f[:])
                            off = 0 if coord == 0 else WGENY
                            nc.sync.dma_start(dbg_fex[:, off:off + w], dfx[:])

            # chain PSUM banks are DVE-zeroed up front and every matmul
            # accumulates (start=False): a start=True mid-stream clobbers
            # any still-open accumulation region sharing the bank on hw
            gorder = list(range(NG))

            def main_batch(b, psa, psb):
                accs = {}
                accBt = psb.tile([128, 2 * IMAGE], f32, name='accB',
                                 tag='accB')
                accs['Bt'] = accBt
                nc.vector.memset(accBt[:], 0.0)
                for yt in range(2):
                    accs[('A', yt)] = psb.tile(
                        [128, 2 * IMAGE], f32, name=f'accA{yt}',
                        tag=f'accA{yt}')
                    nc.vector.memset(accs[('A', yt)][:], 0.0)
                for g in gorder:
                    # per-group window sums + reciprocal (pair layout
                    # col = 32b + 2g + coord) -> MM1 of group g can start
                    # right after its own two activations
                    colx = 2 * NG * b + 2 * g
                    coly = colx + 1
                    nc.vector.reduce_sum(ws_all[:, coly:coly + 1],
                                         fe[(b, g, 1)][:], axis=AX.X)
                    nc.vector.reciprocal(inv_all[:, colx:colx + 2],
                                         ws_all[:, colx:colx + 2])
                    sx = sxs[g]
                    fxn = gp.tile([128, wf], bf16, name='fxn',
                                  tag=f'fxn{g % 2}')
                    nc.vector.tensor_scalar_mul(
                        fxn[:], fe[(b, g, 0)][:, sx:sx + wf],
                        inv_all[:, colx:colx + 1])
                    fy = gp.tile([128, IMAGE], bf16, name=f'fyn{b}_{g}',
                                 tag=f'fyn{b}_{g}')
                    nc.vector.tensor_scalar_mul(
                        fy[:], fe[(b, g, 1)][:, PAD:PAD + IMAGE],
                        inv_all[:, coly:coly + 1])

                    # MM1 -> one [128, 3*wf] PSUM tile, one cast to bf16
                    p1 = psa.tile([128, 3 * wf], f32, name='p1',
                                  tag=f'p1{g % 2}')
                    for c in range(C):
                        nc.tensor.matmul(
                            p1[:, wf * c:wf * c + wf],
                            ps_all[b][:, 384 * g + 128 * c:
                                      384 * g + 128 * c + 128],
                            fxn[:], start=True, stop=True)
                    tt = gp.tile([128, 3 * wf], bf16, name=f'tt{b}_{g}',
                                 tag=f'tt{b}_{g}')
                    if g % 2 == 0:
                        nc.vector.tensor_copy(tt[:], p1[:])
                    else:
                        nc.scalar.copy(tt[:], p1[:])
                    if debug and b == 0 and g == 5:
                        dtt = gp.tile([128, 3 * wf], f32, name='dtt',
                                      tag='dtt')
                        nc.vector.tensor_copy(dtt[:], tt[:])
                        nc.sync.dma_start(dbg_tt[:, 0:3 * wf], dtt[:])
                        nc.sync.dma_start(dbg_ws[:], inv_all[:])

                    # MM2: lhsT fyn slice shared across channels
                    st = False
                    sp = False
                    x0 = x0s[g]
                    for yt in range(2):
                        w = fy[:, 128 * yt:128 * yt + 128]
                        accA = accs[('A', yt)]
                        if windowed:
                            a0 = accA[0:1, x0:x0 + 1]
                            dstA = AP(accA.tensor, a0.offset,
                                      [[2 * IMAGE, 128], [IMAGE, 2], [1, wf]])
                        else:
                            dstA = accA[:]
                        nc.tensor.matmul(dstA, w, tt[:, 0:2 * wf],
                                         start=st, stop=sp,
                                         skip_group_check=True)
                        bcol = IMAGE * yt + x0
                        nc.tensor.matmul(accBt[:, bcol:bcol + wf], w,
                                         tt[:, 2 * wf:3 * wf],
                                         start=st, stop=sp,
                                         skip_group_check=True)
                return accs

            def drain_batch(b, accs):
                for yt in range(2):
                    accA = accs[('A', yt)]
                    obA = gp.tile([128, 2 * IMAGE], f32, name=f'obA{yt}',
                                  tag=f'obA{b}_{yt}')
                    if yt == 0:
                        nc.scalar.mul(obA[:], accA[:], 1.0 / N)
                    else:
                        nc.vector.tensor_scalar_mul(obA[:], accA[:], 1.0 / N)
                    obB = gp.tile([128, IMAGE], f32, name=f'obB{yt}',
                                  tag=f'obB{b}_{yt}')
                    nc.vector.tensor_scalar_mul(
                        obB[:], accs['Bt'][:, IMAGE * yt:IMAGE * (yt + 1)],
                        1.0 / N)
                    ysl = slice(128 * yt, 128 * yt + 128)
                    dmae = [nc.sync, nc.scalar][yt]
                    dmae.dma_start(y_out[b, 0, ysl, :], obA[:, 0:IMAGE])
                    nc.scalar.dma_start(y_out[b, 1, ysl, :],
                                        obA[:, IMAGE:2 * IMAGE])
                    nc.sync.dma_start(y_out[b, 2, ysl, :], obB[:])

            # ---- schedule ----
            gen_acts(0)
            load_patches(1)   # in flight during batch-0 main

            psa_cm = tc.tile_pool(name='ps_a', bufs=1, space='PSUM')
            psa = psa_cm.__enter__()
            psb_cm = tc.tile_pool(name='ps_b', bufs=1, space='PSUM')
            psb = psb_cm.__enter__()
            acc0 = main_batch(0, psa, psb)
            blockdiag(1)
            gen_acts(1)
            drain_batch(0, acc0)
            acc1 = main_batch(1, psa, psb)
            drain_batch(1, acc1)

            psb_cm.__exit__(None, None, None)
            psa_cm.__exit__(None, None, None)

    ndd = _dedupe_ldweights(nc)
    assert ndd >= BLOC * NG, ndd  # most same-weight pairs end up adjacent
    _split_multi_waits(nc)
    _PROGRAMS[windowed] = nc
    return nc


def _norm_np(x):
    mn = x.min(axis=1, keepdims=True)
    mx = x.max(axis=1, keepdims=True)
    return (x - mn) / (mx - mn + EPS)


def _assign_groups(gx):
    """Assign 64 strokes (given gx in [0,256]) to 16 groups of 4 with
    static windows; returns perm (stroke index per slot) or None."""
    lo = np.array([0.0 if X0[g] == 0 else X0[g] + 16.7 for g in range(NG)])
    hi = np.array([256.0 if X0[g] == 128 else X0[g] + 111.3
                   for g in range(NG)])
    order = np.argsort(gx, kind='stable')
    cap = [4] * NG
    slots = [[] for _ in range(NG)]
    for n in order:
        x = gx[n]
        ok = [g for g in range(NG) if cap[g] > 0 and lo[g] <= x <= hi[g]]
        if not ok:
            return None
        g = ok[0]
        cap[g] -= 1
        slots[g].append(n)
    perm = np.concatenate([np.array(s, dtype=np.int64) for s in slots])
    return perm


def _make_in_maps(brushes, patches, windowed, perms):
    import ml_dtypes
    bf = ml_dtypes.bfloat16
    brushes = np.asarray(brushes, dtype=np.float32)
    patches = np.asarray(patches, dtype=np.float32)

    wgen = max((WGENX if windowed else WGENY), WGENY)
    qv = np.arange(128, dtype=np.float32) % 32
    itile = qv[:, None] + np.arange(wgen, dtype=np.float32)[None, :]
    v0s = V0 if windowed else [0] * NG

    in_maps = []
    for k in range(NCORES):
        bsl = brushes[BLOC * k: BLOC * (k + 1)].copy()   # [2, 64, 2]
        psl = patches[BLOC * k: BLOC * (k + 1)].copy()   # [2, 64, 3, 32, 32]
        if windowed:
            for b in range(BLOC):
                p = perms[k * BLOC + b]
                bsl[b] = bsl[b][p]
                psl[b] = psl[b][p]
        # bias table [128, 64]: col = NG*(2b+coord)+g, row = 32*j + q'
        # value = -(coord_of_stroke_4g+j + C - v0_g) * RSIG
        bias = np.zeros((128, 4 * NG), np.float64)
        for b in range(BLOC):
            gx = _norm_np(bsl[b][None, :, 0].astype(np.float64))[0] * IMAGE
            gy = _norm_np(bsl[b][None, :, 1].astype(np.float64))[0] * IMAGE
            for g in range(NG):
                strokes = np.repeat(4 * g + np.arange(4), 32)  # row 32j+q'
                bias[:, NG * 2 * b + g] = -(gx[strokes] + CX
                                            - v0s[g]) * RSIG
                bias[:, NG * (2 * b + 1) + g] = -(gy[strokes] + CY) * RSIG
        pr = psl.reshape(BLOC, NG, 4, C, PH, PW)[..., ::-1, ::-1]
        pt = np.ascontiguousarray(pr.transpose(0, 2, 5, 1, 3, 4)).reshape(
            BLOC, 128, NG * C * PH).astype(bf)
        in_maps.append({'pt_in': pt, 'itile': itile,
                        'bias_in': bias.astype(np.float32)})
    return in_maps


def kernel(brushes: np.ndarray, patches: np.ndarray) -> np.ndarray:
    from concourse.bass_utils import run_bass_kernel_spmd

    brushes = np.asarray(brushes, dtype=np.float32)
    # try windowed group assignment per (core, batch)
    gx_all = _norm_np(brushes[:, :, 0].astype(np.float64)) * IMAGE
    perms = []
    windowed = not os.environ.get('BRUSH_FULL')
    for bi in range(B):
        if not windowed:
            break
        p = _assign_groups(gx_all[bi])
        if p is None:
            windowed = False
            break
        perms.append(p)

    nc = _build_program(windowed)
    in_maps = _make_in_maps(brushes, patches, windowed, perms)
    res = run_bass_kernel_spmd(nc, in_maps, list(range(NCORES)))
    out = np.concatenate([res.results[k]['y_out'] for k in range(NCORES)],
                         axis=0)
    return out
